# revision 1
# baseline (speedup 1.0000x reference)
"""Cached multi-head attention (decode step into a fresh zero cache).

Math: the KV/Q caches are all-zero except slot 0, so the S x S attention
collapses exactly:
  out[b, 0,   h*D+d] = w_bh * v[b,h,d],   w_bh = e^s/(e^s+S-1), s = (q.k)/sqrt(D)
  out[b, s>0, h*D+d] = v[b,h,d] / S
(softmax of an all-zero row is uniform 1/S; only cache row 0 of V is nonzero.)

Sharding: 8 cores = 4 head-groups (3 heads, 192 output cols) x 2 batch-pairs.
Host pre-packs W^T slices in the exact SBUF layout (no on-device transpose).

Device kernel per core, ordered so the V path (which feeds 99.95% of output
bytes) runs first and the bulk output DMAs overlap the Q/K path:
  - V: chunked Wv^T loads -> 6 PE matmuls -> v/S row -> PE outer-product
    broadcast to 128 partitions -> bulk output DMAs (rows 1..2047) with a
    step-0 source AP doing the 16x row replication inside the DMA
  - Q/K: 12 PE matmuls -> score -> w' = S*w -> row 0 DMAs
Input DMAs ride the SP HWDGE ring, output DMAs the ACT ring.
"""

import threading

import numpy as np

B, H, S, D, E = 4, 12, 2048, 64, 768
SCALE = D**-0.5
HG = 3  # heads per core
M = HG * D  # 192 output columns per core
P = 128
NCHUNK = E // P  # 6
JROWS = S // P  # 16 seq rows per partition
N_CORES = 8

XSOFF = 0  # x chunks: [3 types q,k,v][6 chunks][2 batches]
XSEL_COLS = 3 * NCHUNK * 2  # 36

# float32r turned out to require explicit rounding of inputs (it is a
# reduced-precision PE format), so projections stay plain fp32.
USE_F32R = False
WARMUP_MMS = 3  # PE p-state warmup matmuls while input DMAs stream
# (xsel ring, wv piece sizes in e-chunks) — tuned via TimelineSim
DMA_LAYOUT = ("act", (3, 3))

_lock = threading.Lock()
_nc_cache = {}
LAST_RESULTS = None  # BassKernelResults of the most recent run (for test.py)


def _build_nc():
    import concourse.mybir as mybir
    import concourse.tile as tile
    from concourse import bacc
    from concourse.tile import add_dep_helper

    f32 = mybir.dt.float32
    mm_dt = mybir.dt.float32r if USE_F32R else f32
    # Bacc (not Bass): its finalize() runs generate_event_semaphores, which
    # splits multi-sem waits — TRN2 allows only one sync wait per instruction.
    nc = bacc.Bacc("TRN2", target_bir_lowering=False, debug=False)
    wv_d = nc.declare_dram_parameter("wv", [P, NCHUNK * M], f32, isOutput=False)
    wq_d = nc.declare_dram_parameter("wq", [P, NCHUNK * M], f32, isOutput=False)
    wk_d = nc.declare_dram_parameter("wk", [P, NCHUNK * M], f32, isOutput=False)
    xsel_d = nc.declare_dram_parameter("xsel", [P, XSEL_COLS], f32, isOutput=False)
    selm_d = nc.declare_dram_parameter("selm", [2, 2 * P], f32, isOutput=False)
    out = nc.declare_dram_parameter("out", [2, S, M], f32, isOutput=True)

    with tile.TileContext(nc) as tc:
        with (
            tc.tile_pool(name="weights", bufs=1) as w_pool,
            tc.tile_pool(name="small", bufs=1) as small_pool,
            tc.tile_pool(name="vrow", bufs=2) as vrow_pool,
            tc.tile_pool(name="psum_proj", bufs=1, space="PSUM") as psum_proj,
            tc.tile_pool(name="psum_bcast", bufs=2, space="PSUM") as psum_bcast,
        ):
            # ---- V path ----
            # xsel (18KB) carries the matmul lhsT columns; wv is split into
            # staggered pieces so the PE chain starts on the first chunk's
            # semaphore instead of half the tensor's. Ring choice and piece
            # sizes tuned against the cost model (see DMA_LAYOUT).
            xsel_ring, wv_splits = DMA_LAYOUT
            xsel_sb = small_pool.tile([P, XSEL_COLS], f32, tag="xsel")
            (nc.sync if xsel_ring == "sp" else nc.scalar).dma_start(
                xsel_sb[:, :], xsel_d[:, :]
            )
            selm_sb = small_pool.tile([2, 2 * P], f32, tag="selm")
            nc.scalar.dma_start(selm_sb[:, :], selm_d[:, :])
            wv_sb = w_pool.tile([P, NCHUNK * M], f32, tag="wv")
            col = 0
            for nchunks in wv_splits:
                w = nchunks * M
                nc.sync.dma_start(
                    wv_sb[:, col : col + w], wv_d[:, col : col + w]
                )
                col += w

            # PE p-state warmup: keep the tensor engine busy while the wv
            # DMAs stream so the v-projection runs at full clock (cold PE is
            # ~2x slower per matmul). Results are discarded.
            wu = small_pool.tile([P, M], f32, tag="wu")
            nc.vector.memset(wu[:, :], 1.0)
            wu_ps = psum_bcast.tile([P, M], f32, tag="wu_ps")
            last_wu = None
            for _ in range(WARMUP_MMS):
                last_wu = nc.tensor.matmul(
                    wu_ps[:, :], wu[:, 0:P], wu[:, :], start=True, stop=True
                )

            def proj(w_sb, t, tag):
                p_t = psum_proj.tile([2, M], f32, tag=tag)
                first = None
                for c in range(NCHUNK):
                    xcol = XSOFF + t * 2 * NCHUNK + c * 2
                    mm = nc.tensor.matmul(
                        p_t[:, :],
                        xsel_sb[:, xcol : xcol + 2].bitcast(mm_dt),
                        w_sb[:, c * M : (c + 1) * M].bitcast(mm_dt),
                        start=(c == 0),
                        stop=(c == NCHUNK - 1),
                    )
                    if first is None:
                        first = mm
                return p_t, first

            v_ps, v_first = proj(wv_sb, 2, "v")
            if last_wu is not None:
                add_dep_helper(
                    v_first.ins,
                    last_wu.ins,
                    sync=False,
                    reason="warm up PE before the v chain",
                )
            vrow2 = small_pool.tile([2, M], f32, tag="vrow2")
            nc.vector.tensor_scalar_mul(vrow2[:, :], v_ps[:, :], 1.0 / S)

            bcast_mms = []
            for b in range(2):
                # outer product: pb[p, n] = vrow2[b, n] on every partition
                pb = psum_bcast.tile([P, M], f32, tag="bcast")
                mm = nc.tensor.matmul(
                    pb[:, :],
                    selm_sb[:, b * P : (b + 1) * P],
                    vrow2[:, :],
                    start=True,
                    stop=True,
                )
                bcast_mms.append(mm)
                vb = vrow_pool.tile([P, M], f32, tag="vb")
                nc.vector.tensor_copy(vb[:, :], pb[:, :])
                # rows 16..2047: partition p supplies rows 16p..16p+15 via a
                # step-0 (broadcast) source dim; ACT HWDGE ring for outputs
                nc.scalar.dma_start(
                    out[b, JROWS : S, :].rearrange("(p j) m -> p j m", p=P - 1),
                    vb[1:P, :]
                    .rearrange("p (j m) -> p j m", j=1)
                    .broadcast_to([P - 1, JROWS, M]),
                )
                # rows 1..15 from partition 0
                nc.scalar.dma_start(
                    out[b, 1:JROWS, :].rearrange("(p j) m -> p j m", p=1),
                    vb[0:1, :]
                    .rearrange("p (j m) -> p j m", j=1)
                    .broadcast_to([1, JROWS - 1, M]),
                )

            # ---- Q/K path (overlaps the bulk output DMAs above) ----
            wq_sb = w_pool.tile([P, NCHUNK * M], f32, tag="wq")
            nc.sync.dma_start(wq_sb[:, :], wq_d[:, :])
            wk_sb = w_pool.tile([P, NCHUNK * M], f32, tag="wk")
            nc.sync.dma_start(wk_sb[:, :], wk_d[:, :])

            q_ps, q_first = proj(wq_sb, 0, "q")
            k_ps, k_first = proj(wk_sb, 1, "k")
            # keep PE on the V/broadcast path before the Q/K chains
            for mm in (q_first, k_first):
                add_dep_helper(
                    mm.ins,
                    bcast_mms[1].ins,
                    sync=False,
                    reason="broadcast feeds bulk output DMAs; schedule first",
                )
            q_sb = small_pool.tile([2, M], f32, tag="q_sb")
            nc.scalar.copy(q_sb[:, :], q_ps[:, :])
            qk = small_pool.tile([2, M], f32, tag="qk")
            nc.vector.tensor_mul(qk[:, :], q_sb[:, :], k_ps[:, :])
            s3 = small_pool.tile([2, HG], f32, tag="s3")
            nc.vector.tensor_reduce(
                s3[:, :],
                qk[:, :].rearrange("p (h d) -> p h d", d=D),
                axis=mybir.AxisListType.X,
                op=mybir.AluOpType.add,
            )
            # w' = S*w = 1/(((S-1)/S)*exp(-s*SCALE) + 1/S)
            t3 = small_pool.tile([2, HG], f32, tag="t3")
            nc.scalar.activation(
                t3[:, :], s3[:, :], mybir.ActivationFunctionType.Exp, scale=-SCALE
            )
            u3 = small_pool.tile([2, HG], f32, tag="u3")
            nc.vector.tensor_scalar(
                u3[:, :],
                t3[:, :],
                float(S - 1) / S,
                1.0 / S,
                mybir.AluOpType.mult,
                mybir.AluOpType.add,
            )
            w2 = small_pool.tile([2, HG], f32, tag="w2")
            nc.vector.reciprocal(w2[:, :], u3[:, :])
            # row 0 = (v/S) * w' = v * w, per head
            row0 = small_pool.tile([2, M], f32, tag="row0")
            for h in range(HG):
                nc.vector.tensor_scalar_mul(
                    row0[:, h * D : (h + 1) * D],
                    vrow2[:, h * D : (h + 1) * D],
                    w2[:, h : h + 1],
                )
            # both batches' row 0 in one DMA: [2, 1, 192] is 3 AP dims
            nc.sync.dma_start(
                out[0:2, 0:1, :],
                row0[:, :].rearrange("p (j m) -> p j m", j=1),
            )
    nc.finalize()
    return nc


def _get_nc():
    with _lock:
        if "nc" not in _nc_cache:
            _nc_cache["nc"] = _build_nc()
        return _nc_cache["nc"]


def _prep_w(W, g):
    # W: [H, D, E] -> [128, NCHUNK*M] with element (p, c*M+m) = W[3g+m//D, m%D, c*128+p]
    sl = np.asarray(W, dtype=np.float32)[HG * g : HG * (g + 1)].reshape(M, E)
    return np.ascontiguousarray(
        sl.T.reshape(NCHUNK, P, M).transpose(1, 0, 2).reshape(P, NCHUNK * M)
    )


def _prep_x(x2):
    # x2: [2, E] -> [128, NCHUNK*2] with element (p, c*2+b) = x2[b, c*128+p]
    t = np.asarray(x2, dtype=np.float32).reshape(2, NCHUNK, P)
    return np.ascontiguousarray(t.transpose(2, 1, 0).reshape(P, NCHUNK * 2))


def kernel(query, key, value, Wq, Wk, Wv):
    global LAST_RESULTS
    from concourse.bass_utils import run_bass_kernel_spmd

    query = np.asarray(query, dtype=np.float32).reshape(B, E)
    key = np.asarray(key, dtype=np.float32).reshape(B, E)
    value = np.asarray(value, dtype=np.float32).reshape(B, E)

    sel = np.zeros((2, 2 * P), dtype=np.float32)
    sel[0, 0:P] = 1.0
    sel[1, P : 2 * P] = 1.0

    in_maps = []
    for c in range(N_CORES):
        g, bp = c % 4, c // 4
        xs = np.concatenate(
            [
                _prep_x(query[2 * bp : 2 * bp + 2]),
                _prep_x(key[2 * bp : 2 * bp + 2]),
                _prep_x(value[2 * bp : 2 * bp + 2]),
            ],
            axis=1,
        )
        in_maps.append(
            {
                "wv": _prep_w(Wv, g),
                "wq": _prep_w(Wq, g),
                "wk": _prep_w(Wk, g),
                "xsel": np.ascontiguousarray(xs),
                "selm": sel,
            }
        )

    nc = _get_nc()
    LAST_RESULTS = run_bass_kernel_spmd(nc, in_maps, core_ids=list(range(N_CORES)))
    res = LAST_RESULTS.results

    full = np.empty((B, S, H * D), dtype=np.float32)
    for c in range(N_CORES):
        g, bp = c % 4, c // 4
        full[2 * bp : 2 * bp + 2, :, M * g : M * (g + 1)] = res[c]["out"]
    return full

